# revision 1
# baseline (speedup 1.0000x reference)
"""Additive (Bahdanau-style) attention scores kernel for Trainium2.

Computes softmax(We @ tanh(query@Wq.T + keys@Wk.T), axis=-1) for
B=32, S=2048, D=1024, data-parallel over batch across 8 NeuronCores.

Per-core strategy (v6, fp16 datapath, DRAM-staged fp16 keys):
  - weights arrive pre-transposed and fp16 from the host (replicated,
    tiny); keys are cast f32->fp16 during the SWDGE load, then
    transposed to [d, s] by the XBAR DMA-transpose engine
  - pkT[e, s] = WkT[d,e].T @ keysT[d,s] in fp16 (full PE rate, fp32 acc)
  - energy = tanh(pkT + pq[e]) on ACT with per-partition bias
  - scores[s] = sum_e We[e] * energy[e, s] as rank-1 accumulating
    matmuls, deferred into the NEXT block's pk stream so the PE never
    waits on ACT
  - per-batch softmax over s (DVE/ACT), overlapped with the main loop
"""

import numpy as np
from contextlib import ExitStack

import concourse.bass as bass
import concourse.mybir as mybir
import concourse.tile as tile
from concourse import bacc
from concourse.bass_utils import run_bass_kernel_spmd
from concourse.masks import make_identity
from concourse.tile_rust import add_dep_helper

f32 = mybir.dt.float32
fp16 = mybir.dt.float16

B, S, D, E = 32, 2048, 1024, 1024
NCORES = 8
BL = B // NCORES      # 4 batches per core
S_BLK = 512
N_SBLK = S // S_BLK   # 4
DT = D // 128         # 8 d-tiles
ET = E // 128         # 8 e-tiles
ST = S_BLK // 128     # 4 s-subtiles per block

_CACHE: dict = {}


def _build_nc():
    nc = bacc.Bacc("TRN2", target_bir_lowering=False, debug=False, num_devices=NCORES)

    keys_d = nc.dram_tensor("keys", [BL, S, D], f32, kind="ExternalInput")
    keys16_d = nc.dram_tensor("keys16", [BL, S, D], fp16, kind="Internal")
    qT_d = nc.dram_tensor("queryT16", [128, DT, BL], fp16, kind="ExternalInput")
    wkT_d = nc.dram_tensor("wkT16", [128, DT, E], fp16, kind="ExternalInput")
    wqT_d = nc.dram_tensor("wqT16", [128, DT, E], fp16, kind="ExternalInput")
    weT_d = nc.dram_tensor("weT16", [128, ET], fp16, kind="ExternalInput")
    out_d = nc.dram_tensor("out", [BL, S], f32, kind="ExternalOutput")

    with tile.TileContext(nc) as tc, ExitStack() as ctx:
        wpool = ctx.enter_context(tc.tile_pool(name="weights", bufs=1))
        kT_pool = ctx.enter_context(tc.tile_pool(name="kT", bufs=3))
        en_pool = ctx.enter_context(tc.tile_pool(name="en", bufs=12))
        small = ctx.enter_context(tc.tile_pool(name="small", bufs=1))
        sm_pool = ctx.enter_context(tc.tile_pool(name="smx", bufs=2))

        ps_pk = ctx.enter_context(tc.tile_pool(name="ps_pk", bufs=3, space="PSUM"))
        ps_sc = ctx.enter_context(tc.tile_pool(name="ps_sc", bufs=2, space="PSUM"))
        ps_pq = ctx.enter_context(tc.tile_pool(name="ps_pq", bufs=1, space="PSUM"))

        # ---- weights: packed fp16 HWDGE loads on the scalar queue ----
        # (Sync queue is reserved for the kT XBAR transposes)
        qT_sb = wpool.tile([128, DT, BL], fp16)
        nc.scalar.dma_start(qT_sb, qT_d[:])
        wqT_sb = wpool.tile([128, DT, E], fp16)
        nc.scalar.dma_start(wqT_sb, wqT_d[:])
        wkT_sb = wpool.tile([128, DT, E], fp16)
        wkT_load = nc.scalar.dma_start(wkT_sb, wkT_d[:])
        weT_sb = wpool.tile([128, ET], fp16)
        nc.scalar.dma_start(weT_sb, weT_d[:])

        # ---- pq: layout-A matmul [b, e], then tiny PE transposes to [e, b] ----
        ident4 = wpool.tile([BL, BL], fp16)
        make_identity(nc, ident4)
        pq_row = wpool.tile([BL, E], fp16)
        for half in range(2):
            pq_ps = ps_pq.tile([BL, 512], f32, tag="pq_mm")
            for dt in range(DT):
                nc.tensor.matmul(pq_ps,
                                 lhsT=qT_sb[:, dt],
                                 rhs=wqT_sb[:, dt, half * 512 : (half + 1) * 512],
                                 start=(dt == 0), stop=(dt == DT - 1))
            nc.vector.tensor_copy(pq_row[:, half * 512 : (half + 1) * 512], pq_ps)
        pq_sb = wpool.tile([128, ET, BL], fp16)
        for et in range(ET):
            pq_tp = ps_pq.tile([128, BL], fp16, tag="pq_tr")
            nc.tensor.transpose(pq_tp, pq_row[:, et * 128 : (et + 1) * 128], ident4)
            nc.vector.tensor_copy(pq_sb[:, et], pq_tp)

        # engine writes must start at partition 0, so scores accumulate on
        # partition 0 (one row per core) and per-batch softmax reads slices
        scores_tmp = small.tile([1, BL * S], f32)

        def emit_softmax(b):
            """Softmax of batch b over scores_tmp[0, b*S:(b+1)*S] -> out."""
            row = scores_tmp[0:1, b * S : (b + 1) * S]
            mx = sm_pool.tile([1, 1], f32, tag="mx")
            nc.vector.reduce_max(mx, row, axis=mybir.AxisListType.X)
            neg_mx = sm_pool.tile([1, 1], f32, tag="negmx")
            nc.vector.tensor_scalar_mul(neg_mx, mx, -1.0)
            ex = sm_pool.tile([1, S], f32, tag="ex")
            sumx = sm_pool.tile([1, 1], f32, tag="sumx")
            nc.scalar.activation(ex, row, mybir.ActivationFunctionType.Exp,
                                 bias=neg_mx, scale=1.0, accum_out=sumx)
            rinv = sm_pool.tile([1, 1], f32, tag="rinv")
            nc.vector.reciprocal(rinv, sumx)
            outr = sm_pool.tile([1, S], f32, tag="outr")
            nc.vector.tensor_scalar_mul(outr, ex, rinv)
            nc.scalar.dma_start(out_d[b : b + 1, :], outr)

        # ---- main loop over (batch, s-block) ----
        pending = None  # deferred We-contraction of the previous block

        blocks = [(b, sblk) for b in range(BL) for sblk in range(N_SBLK)]
        # group blocks for DRAM-staged fp16 keys: small groups first for a
        # fast ramp, then 2-block groups (few, big serialized DMA ops)
        groups = [[0], [1]] + [[i, i + 1] for i in range(2, len(blocks), 2)]
        blk_group = {}
        for gi, g in enumerate(groups):
            for bi_ in g:
                blk_group[bi_] = gi
        kT_tiles = {}
        last_xbar = wkT_load

        def emit_group(gi):
            g = groups[gi]
            bi0, (b0_, sblk0_) = g[0], blocks[g[0]]
            flat0 = b0_ * S + sblk0_ * S_BLK
            n_s = S_BLK * len(g)
            flat = keys_d[:].rearrange("b s d -> (b s) d")
            flat16 = keys16_d[:].rearrange("b s d -> (b s) d")
            c = nc.gpsimd.dma_start(flat16[flat0 : flat0 + n_s, :],
                                    flat[flat0 : flat0 + n_s, :])
            if last_xbar is not None:
                add_dep_helper(c.ins, last_xbar.ins, sync=True,
                               reason="batch DMA modes")
            kT = kT_pool.tile([128, DT, n_s], fp16,
                              tag=f"kT_{len(g)}")
            x = nc.sync.dma_start_transpose(kT, flat16[flat0 : flat0 + n_s, :])
            for off_, bi_ in enumerate(g):
                kT_tiles[bi_] = (kT, off_ * S_BLK)
            return x

        emit_group(0)
        last_xbar = emit_group(1)

        for bi, (b, sblk) in enumerate(blocks):
            if bi + 2 < len(blocks):
                gi = blk_group[bi + 2]
                if min(groups[gi]) == bi + 2:
                    last_xbar = emit_group(gi)
            kT, s_off = kT_tiles.pop(bi)

            en_tiles = []
            for et in range(ET):
                pk_ps = ps_pk.tile([128, S_BLK], f32)
                for dt in range(DT):
                    nc.tensor.matmul(
                        pk_ps,
                        lhsT=wkT_sb[:, dt, et * 128 : (et + 1) * 128],
                        rhs=kT[:, dt, s_off : s_off + S_BLK],
                        start=(dt == 0), stop=(dt == DT - 1))
                en = en_pool.tile([128, S_BLK], fp16)
                nc.scalar.activation(en, pk_ps,
                                     mybir.ActivationFunctionType.Tanh,
                                     bias=pq_sb[:, et, b : b + 1],
                                     scale=1.0)
                en_tiles.append(en)
                if et == 1 and pending is not None:
                    pending()
                    pending = None

            def make_pending(b_, sblk_, tiles):
                def emit():
                    sc_ps = ps_sc.tile([1, S_BLK], f32)
                    for et_ in range(ET):
                        nc.tensor.matmul(sc_ps,
                                         lhsT=weT_sb[:, et_ : et_ + 1],
                                         rhs=tiles[et_],
                                         start=(et_ == 0), stop=(et_ == ET - 1),
                                         skip_group_check=True)
                    off = b_ * S + sblk_ * S_BLK
                    nc.vector.tensor_copy(
                        scores_tmp[0 : 1, off : off + S_BLK], sc_ps)
                    if sblk_ == N_SBLK - 1:
                        emit_softmax(b_)
                return emit

            pending = make_pending(b, sblk, en_tiles)

        pending()

    nc.compile()
    return nc


def _get_nc():
    if "nc" not in _CACHE:
        _CACHE["nc"] = _build_nc()
    return _CACHE["nc"]


def kernel(query, keys, Wq, Wk, We, _return_raw=False, _trace=False):
    query = np.asarray(query, dtype=np.float32)
    keys = np.asarray(keys, dtype=np.float32)
    Wq = np.asarray(Wq, dtype=np.float32)
    Wk = np.asarray(Wk, dtype=np.float32)
    We = np.asarray(We, dtype=np.float32)

    # pack [D, E] -> [128(p), DT, E] with d = dt*128 + p, fp16
    wkT = np.ascontiguousarray(
        Wk.T.reshape(DT, 128, E).transpose(1, 0, 2)).astype(np.float16)
    wqT = np.ascontiguousarray(
        Wq.T.reshape(DT, 128, E).transpose(1, 0, 2)).astype(np.float16)
    weT = np.ascontiguousarray(
        We.reshape(ET, 128).T).astype(np.float16)

    in_maps = []
    for c in range(NCORES):
        bs = slice(c * BL, (c + 1) * BL)
        in_maps.append({
            "keys": np.ascontiguousarray(keys[bs]),
            "queryT16": np.ascontiguousarray(
                query[bs].T.reshape(DT, 128, BL).transpose(1, 0, 2)
            ).astype(np.float16),
            "wkT16": wkT,
            "wqT16": wqT,
            "weT16": weT,
        })

    nc = _get_nc()
    res = run_bass_kernel_spmd(nc, in_maps, list(range(NCORES)), trace=_trace)
    out = np.concatenate([res.results[c]["out"] for c in range(NCORES)], axis=0)
    if _return_raw:
        return out, res
    return out



# revision 2
# speedup vs baseline: 1.3546x; 1.3546x over previous
"""Additive (Bahdanau-style) attention scores kernel for Trainium2.

Computes softmax(We @ tanh(query@Wq.T + keys@Wk.T), axis=-1) for
B=32, S=2048, D=1024, data-parallel over batch across 8 NeuronCores.

Per-core strategy (v7, fp8 DoubleRow datapath, DRAM-staged fp8 keys):
  - keys are cast f32->fp8e4 during an SWDGE DRAM->DRAM staging DMA,
    then transposed by the XBAR viewing fp8 byte-pairs as u16: the
    transposed tile kT16[p, c, s] holds keys8[s, 2*(c*128+p)+{0,1}]
  - pk matmuls run in fp8 DoubleRow (2 contraction rows/cell): each
    [128,512] psum tile accumulates 4 passes (cp, j) of 256 d each,
    with the ifmap AP [128, Ko=2 (c-pair, big stride), s (stride 2B)]
    at byte offset j; Wk is host-packed (x64 scale) to match
  - energy = tanh(pk/64 + pq[e]) on ACT with per-partition bias, fp16
  - scores[s] = sum_e We[e] * energy[e, s] as rank-1 fp16 accumulating
    matmuls, deferred into the NEXT block's pk stream
  - per-batch softmax over s (DVE/ACT), overlapped with the main loop
"""

import numpy as np
import ml_dtypes
from contextlib import ExitStack

import concourse.bass as bass
import concourse.mybir as mybir
import concourse.tile as tile
from concourse import bacc
from concourse.bass_utils import run_bass_kernel_spmd
from concourse.masks import make_identity
from concourse.tile_rust import add_dep_helper

f32 = mybir.dt.float32
fp16 = mybir.dt.float16
fp8 = mybir.dt.float8e4
u16 = mybir.dt.uint16
E4 = ml_dtypes.float8_e4m3

B, S, D, E = 32, 2048, 1024, 1024
NCORES = 8
BL = B // NCORES      # 4 batches per core
S_BLK = 512
N_SBLK = S // S_BLK   # 4
DT = D // 128         # 8 d-tiles
ET = E // 128         # 8 e-tiles
CT = D // 256         # 4 u16-column tiles (c)
WK_SCALE = 64.0

_CACHE: dict = {}


def _build_nc():
    nc = bacc.Bacc("TRN2", target_bir_lowering=False, debug=False, num_devices=NCORES)

    keys_d = nc.dram_tensor("keys", [BL, S, D], f32, kind="ExternalInput")
    keys8_d = nc.dram_tensor("keys8", [BL, S, D], fp8, kind="Internal")
    qT_d = nc.dram_tensor("queryT16", [128, DT, BL], fp16, kind="ExternalInput")
    # wk8: [p, cp, j, i, e] fp8, = Wk[e, d]*64 with d = 2*((2*cp+i)*128+p)+j
    wk8_d = nc.dram_tensor("wk8", [128, 2, 2, 2, E], fp8, kind="ExternalInput")
    wqT_d = nc.dram_tensor("wqT16", [128, DT, E], fp16, kind="ExternalInput")
    weT_d = nc.dram_tensor("weT16", [128, ET], fp16, kind="ExternalInput")
    out_d = nc.dram_tensor("out", [BL, S], f32, kind="ExternalOutput")

    with tile.TileContext(nc) as tc, ExitStack() as ctx:
        wpool = ctx.enter_context(tc.tile_pool(name="weights", bufs=1))
        kT_pool = ctx.enter_context(tc.tile_pool(name="kT", bufs=3))
        en_pool = ctx.enter_context(tc.tile_pool(name="en", bufs=12))
        small = ctx.enter_context(tc.tile_pool(name="small", bufs=1))
        sm_pool = ctx.enter_context(tc.tile_pool(name="smx", bufs=2))

        ps_pk = ctx.enter_context(tc.tile_pool(name="ps_pk", bufs=3, space="PSUM"))
        ps_sc = ctx.enter_context(tc.tile_pool(name="ps_sc", bufs=2, space="PSUM"))
        ps_pq = ctx.enter_context(tc.tile_pool(name="ps_pq", bufs=1, space="PSUM"))

        # ---- weights: packed HWDGE loads on the scalar queue ----
        qT_sb = wpool.tile([128, DT, BL], fp16)
        nc.scalar.dma_start(qT_sb, qT_d[:])
        wqT_sb = wpool.tile([128, DT, E], fp16)
        nc.scalar.dma_start(wqT_sb, wqT_d[:])
        wk8_sb = wpool.tile([128, 2, 2, 2, E], fp8)
        wk8_load = nc.scalar.dma_start(wk8_sb, wk8_d[:])
        weT_sb = wpool.tile([128, ET], fp16)
        nc.scalar.dma_start(weT_sb, weT_d[:])

        # ---- pq: layout-A matmul [b, e], then tiny PE transposes to [e, b] ----
        ident4 = wpool.tile([BL, BL], fp16)
        make_identity(nc, ident4)
        pq_row = wpool.tile([BL, E], fp16)
        for half in range(2):
            pq_ps = ps_pq.tile([BL, 512], f32, tag="pq_mm")
            for dt in range(DT):
                nc.tensor.matmul(pq_ps,
                                 lhsT=qT_sb[:, dt],
                                 rhs=wqT_sb[:, dt, half * 512 : (half + 1) * 512],
                                 start=(dt == 0), stop=(dt == DT - 1))
            nc.vector.tensor_copy(pq_row[:, half * 512 : (half + 1) * 512], pq_ps)
        pq_sb = wpool.tile([128, ET, BL], fp16)
        for et in range(ET):
            pq_tp = ps_pq.tile([128, BL], fp16, tag="pq_tr")
            nc.tensor.transpose(pq_tp, pq_row[:, et * 128 : (et + 1) * 128], ident4)
            nc.vector.tensor_copy(pq_sb[:, et], pq_tp)

        # engine writes must start at partition 0, so scores accumulate on
        # partition 0 (one row per core) and per-batch softmax reads slices
        scores_tmp = small.tile([1, BL * S], f32)

        def emit_softmax(b):
            """Softmax of batch b over scores_tmp[0, b*S:(b+1)*S] -> out."""
            row = scores_tmp[0:1, b * S : (b + 1) * S]
            mx = sm_pool.tile([1, 1], f32, tag="mx")
            nc.vector.reduce_max(mx, row, axis=mybir.AxisListType.X)
            neg_mx = sm_pool.tile([1, 1], f32, tag="negmx")
            nc.vector.tensor_scalar_mul(neg_mx, mx, -1.0)
            ex = sm_pool.tile([1, S], f32, tag="ex")
            sumx = sm_pool.tile([1, 1], f32, tag="sumx")
            nc.scalar.activation(ex, row, mybir.ActivationFunctionType.Exp,
                                 bias=neg_mx, scale=1.0, accum_out=sumx)
            rinv = sm_pool.tile([1, 1], f32, tag="rinv")
            nc.vector.reciprocal(rinv, sumx)
            outr = sm_pool.tile([1, S], f32, tag="outr")
            nc.vector.tensor_scalar_mul(outr, ex, rinv)
            nc.scalar.dma_start(out_d[b : b + 1, :], outr)

        # ---- main loop over (batch, s-block) ----
        pending = None  # deferred We-contraction of the previous block

        blocks = [(b, sblk) for b in range(BL) for sblk in range(N_SBLK)]
        # group blocks for DRAM-staged fp8 keys: small groups first for a
        # fast ramp, then 2-block groups (few, big serialized DMA ops)
        groups = [[0], [1]] + [[i, i + 1] for i in range(2, len(blocks), 2)]
        blk_group = {}
        for gi, g in enumerate(groups):
            for bi_ in g:
                blk_group[bi_] = gi
        kT_tiles = {}
        last_xbar = wk8_load

        def emit_group(gi):
            g = groups[gi]
            bi0, (b0_, sblk0_) = g[0], blocks[g[0]]
            flat0 = b0_ * S + sblk0_ * S_BLK
            n_s = S_BLK * len(g)
            flat = keys_d[:].rearrange("b s d -> (b s) d")
            flat8 = keys8_d[:].rearrange("b s d -> (b s) d")
            c = nc.gpsimd.dma_start(flat8[flat0 : flat0 + n_s, :],
                                    flat[flat0 : flat0 + n_s, :])
            if last_xbar is not None:
                add_dep_helper(c.ins, last_xbar.ins, sync=True,
                               reason="batch DMA modes")
            kT = kT_pool.tile([128, CT, n_s], u16, tag=f"kT_{len(g)}")
            x = nc.sync.dma_start_transpose(
                kT, flat8[flat0 : flat0 + n_s, :].bitcast(u16))
            for off_, bi_ in enumerate(g):
                kT_tiles[bi_] = (kT, off_ * S_BLK)
            return x

        emit_group(0)
        last_xbar = emit_group(1)

        for bi, (b, sblk) in enumerate(blocks):
            if bi + 2 < len(blocks):
                gi = blk_group[bi + 2]
                if min(groups[gi]) == bi + 2:
                    last_xbar = emit_group(gi)
            kT, s_off = kT_tiles.pop(bi)
            # fp8 byte view: [128, CT, n_s, 2]
            kT8 = kT[:].bitcast(fp8).rearrange("p c (s j) -> p c s j", j=2)

            en_tiles = []
            for et in range(ET):
                pk_ps = ps_pk.tile([128, S_BLK], f32)
                for idx, (cp, j) in enumerate(
                        [(c_, j_) for c_ in range(2) for j_ in range(2)]):
                    nc.tensor.matmul(
                        pk_ps,
                        lhsT=wk8_sb[:, cp, j, :, et * 128 : (et + 1) * 128],
                        rhs=kT8[:, 2 * cp : 2 * cp + 2,
                                s_off : s_off + S_BLK, j],
                        start=(idx == 0), stop=(idx == 3),
                        perf_mode=mybir.MatmulPerfMode.DoubleRow)
                en = en_pool.tile([128, S_BLK], fp16)
                nc.scalar.activation(en, pk_ps,
                                     mybir.ActivationFunctionType.Tanh,
                                     bias=pq_sb[:, et, b : b + 1],
                                     scale=1.0 / WK_SCALE)
                en_tiles.append(en)
                if et == 1 and pending is not None:
                    pending()
                    pending = None

            def make_pending(b_, sblk_, tiles):
                def emit():
                    sc_ps = ps_sc.tile([1, S_BLK], f32)
                    for et_ in range(ET):
                        nc.tensor.matmul(sc_ps,
                                         lhsT=weT_sb[:, et_ : et_ + 1],
                                         rhs=tiles[et_],
                                         start=(et_ == 0), stop=(et_ == ET - 1),
                                         skip_group_check=True)
                    off = b_ * S + sblk_ * S_BLK
                    nc.vector.tensor_copy(
                        scores_tmp[0 : 1, off : off + S_BLK], sc_ps)
                    if sblk_ == N_SBLK - 1:
                        emit_softmax(b_)
                return emit

            pending = make_pending(b, sblk, en_tiles)

        pending()

    nc.compile()
    return nc


def _get_nc():
    if "nc" not in _CACHE:
        _CACHE["nc"] = _build_nc()
    return _CACHE["nc"]


def kernel(query, keys, Wq, Wk, We, _return_raw=False, _trace=False):
    query = np.asarray(query, dtype=np.float32)
    keys = np.asarray(keys, dtype=np.float32)
    Wq = np.asarray(Wq, dtype=np.float32)
    Wk = np.asarray(Wk, dtype=np.float32)
    We = np.asarray(We, dtype=np.float32)

    # wk8[p, cp, j, i, e] = 64*Wk[e, d], d = 2*((2*cp+i)*128+p)+j
    # d = 2*(c*128+p)+j with c = 2*cp+i; build via reshape:
    # Wk.T is [d, e]; index d as (c, p, j): d = 2*(c*128+p)+j -> [c, p, j]
    wk_scaled = (Wk.T * WK_SCALE).astype(np.float32)  # [d, e]
    wk_cpje = wk_scaled.reshape(CT, 128, 2, E)        # [c, p, j, e]
    # -> [p, cp, j, i, e] with c = 2*cp+i
    wk8 = np.ascontiguousarray(
        wk_cpje.reshape(2, 2, 128, 2, E)              # [cp, i, p, j, e]
        .transpose(2, 0, 3, 1, 4)                     # [p, cp, j, i, e]
    ).astype(E4)

    wqT = np.ascontiguousarray(
        Wq.T.reshape(DT, 128, E).transpose(1, 0, 2)).astype(np.float16)
    weT = np.ascontiguousarray(
        We.reshape(ET, 128).T).astype(np.float16)

    in_maps = []
    for c in range(NCORES):
        bs = slice(c * BL, (c + 1) * BL)
        in_maps.append({
            "keys": np.ascontiguousarray(keys[bs]),
            "queryT16": np.ascontiguousarray(
                query[bs].T.reshape(DT, 128, BL).transpose(1, 0, 2)
            ).astype(np.float16),
            "wk8": wk8,
            "wqT16": wqT,
            "weT16": weT,
        })

    nc = _get_nc()
    res = run_bass_kernel_spmd(nc, in_maps, list(range(NCORES)), trace=_trace)
    out = np.concatenate([res.results[c]["out"] for c in range(NCORES)], axis=0)
    if _return_raw:
        return out, res
    return out
